# revision 8
# baseline (speedup 1.0000x reference)
"""CPMAnt attention kernel for 8 TRN2 NeuronCores.

Sharding: tensor-parallel over heads. Each core computes 4 of the 32 heads:
q/k/v projections with column-sliced Wq/Wk/Wv, attention with its slice of
position_bias, and a partial output projection with the row-sliced Wo. The 8
partial outputs are summed on the host (the all-reduce).

Attention is computed entirely in the transposed layout, which removes the
PE transposes and most DVE traffic of the usual [q,k]-layout softmax:

  qT/kT [dh, r]  = Wx^T-slice @ hidden^T        (lhsT = W tiles, rhs = hT)
  v     [r, dh]  = hidden @ Wv-slice
  sT    [k, q]   = kT-tile.T @ qT               (scoresT, one mm per k-tile)
  pT    [k, q]   = exp(sT - ln256) * ebT        (Act exp; DVE mult)
  rowsum[1, q]   = ones128.T @ tree_sum(pT)     (DVE adds + one PE mm)
  ctxU  [dh, q]  = v-tiles.T @ pT               (unnormalized context)
  ctxT  [dh, q]  = ctxU * (ones ⊗ 1/rowsum)     (PE outer-product + DVE mult)
  outT  [D, r]   = Wo-slice tiles.T @ ctxT      (partial, summed on host)

ebT = exp(position_bias)·mask is precomputed on the host, so the additive
bias+mask becomes a single fp16 multiply: exp(s+b) = exp(s-ln256)·exp(b)·256
with the 2^-8 shift cancelling in the (deferred) softmax normalization and
keeping every fp16 intermediate in range (probs<~12, rowsums<~16k).
Masked entries multiply by exactly 0, reproducing the reference's
post-softmax mask zeroing.
"""

import math

import numpy as np

B, S, D = 2, 1024, 4096
H, DH = 32, 128
NCORES = 8
HPC = H // NCORES  # heads per core = 4
R = B * S  # 2048 rows
KT = D // 128  # 32 contraction tiles for the projections
NB = R // 512  # 4 row blocks
SCALE = 1.0 / math.sqrt(DH)
SHIFT = math.log(256.0)


def _build_core_kernel(repeat: int = 1):
    import concourse.mybir as mybir
    from concourse import bacc
    from concourse.tile import TileContext

    f32 = mybir.dt.float32
    fp16 = mybir.dt.float16
    Exp = mybir.ActivationFunctionType.Exp

    nc = bacc.Bacc("TRN2")

    hqT = nc.declare_dram_parameter("hqT", [D, R], fp16, isOutput=False)
    hkvT = nc.declare_dram_parameter("hkvT", [D, R], fp16, isOutput=False)
    wq = nc.declare_dram_parameter("wq", [D, 512], fp16, isOutput=False)
    wk = nc.declare_dram_parameter("wk", [D, 512], fp16, isOutput=False)
    wv = nc.declare_dram_parameter("wv", [D, 512], fp16, isOutput=False)
    wo = nc.declare_dram_parameter("wo", [512, D], fp16, isOutput=False)
    eb = nc.declare_dram_parameter("eb", [NB, HPC, 128, KT * 128], fp16, isOutput=False)
    outp = nc.declare_dram_parameter("outp", [NB, 128, KT, 512], fp16, isOutput=True)

    hq3 = hqT.rearrange("(t p) r -> p t r", p=128)  # [128, 32, 2048]
    hkv3 = hkvT.rearrange("(t p) r -> p t r", p=128)
    wq3 = wq.rearrange("(t p) m -> p t m", p=128)  # [128, 32, 512]
    wk3 = wk.rearrange("(t p) m -> p t m", p=128)
    wv3 = wv.rearrange("(t p) m -> p t m", p=128)
    wo3 = wo.rearrange("(t p) m -> p t m", p=128)  # [128, 4, 4096]

    with TileContext(nc) as tc:
      for _rep in range(repeat):
        with (
            tc.tile_pool(name="pers", bufs=1) as pers,
            tc.tile_pool(name="ebp", bufs=2) as ebp,
            tc.tile_pool(name="ppool", bufs=3) as ppool,
            tc.tile_pool(name="epool", bufs=4) as epool,
            tc.tile_pool(name="tpool", bufs=2) as tpool,
            tc.tile_pool(name="apool", bufs=2) as apool,
            tc.tile_pool(name="rpool", bufs=2) as rpool,
            tc.tile_pool(name="opool", bufs=2) as opool,
        ):
            qT_s = pers.tile([128, HPC, R], fp16)  # 16KB/part
            kT_s = pers.tile([128, HPC, R], fp16)  # 16KB/part
            v_s = pers.tile([128, 16, 512], fp16)  # 16KB/part
            ones_col = pers.tile([128, 1], fp16)
            ones_row = pers.tile([1, 128], fp16)
            shift_b = pers.tile([128, 1], f32)
            nc.vector.memset(ones_col, 1.0)
            nc.vector.memset(ones_row, 1.0)
            nc.vector.memset(shift_b, -SHIFT)

            eb_tiles = {}

            def eb_dma(half):  # half = n*2 + h//2, covers 2 heads
                n, hh = divmod(half, 2)
                t = ebp.tile([128, 2, 8, 512], fp16, tag="eb", name="ebt")
                nc.sync.dma_start(
                    out=t,
                    in_=eb[n, hh * 2 : (hh + 1) * 2].rearrange(
                        "h p (k c) -> p h k c", k=8
                    ),
                )
                eb_tiles[half] = t

            # ---------------- Phase A: projections ----------------
            with (
                tc.tile_pool(name="wstage", bufs=4) as wpool,
                tc.tile_pool(name="hstream", bufs=2) as hpool,
                tc.tile_pool(name="projpsum", bufs=2, space="PSUM") as pp,
            ):

                def stage_w(w3):
                    whs = []
                    for qt in range(8):
                        wh = wpool.tile([128, 4, 512], fp16, tag=f"W{qt}", name="wh")
                        nc.gpsimd.dma_start(out=wh, in_=w3[:, qt * 4 : (qt + 1) * 4, :])
                        whs.append(wh)
                    return whs

                def proj_qk(whs, hsrc3, dst, scale):
                    for n in range(NB):
                        psums = [
                            pp.tile([128, 512], f32, tag=f"pp{m}", name=f"pp{m}")
                            for m in range(4)
                        ]
                        for ktg in range(4):
                            ht = hpool.tile([128, 8, 512], fp16, tag="ht", name="ht")
                            nc.sync.dma_start(
                                out=ht,
                                in_=hsrc3[
                                    :, ktg * 8 : (ktg + 1) * 8, n * 512 : (n + 1) * 512
                                ],
                            )
                            for kl in range(8):
                                kt = ktg * 8 + kl
                                for m in range(4):
                                    nc.tensor.matmul(
                                        psums[m],
                                        whs[kt // 4][:, kt % 4, m * 128 : (m + 1) * 128],
                                        ht[:, kl, :],
                                        start=(kt == 0),
                                        stop=(kt == KT - 1),
                                    )
                        for m in range(4):
                            o = dst[:, m, n * 512 : (n + 1) * 512]
                            if m % 2 == 0:
                                nc.scalar.mul(out=o, in_=psums[m], mul=scale)
                            else:
                                nc.vector.tensor_scalar_mul(
                                    out=o, in0=psums[m], scalar1=scale
                                )

                whs = stage_w(wq3)
                proj_qk(whs, hq3, qT_s, SCALE)
                whs = stage_w(wk3)
                proj_qk(whs, hkv3, kT_s, 1.0)

                # v projection: v[r, c] += hT[kt, r].T @ Wv[kt, c]
                whs = stage_w(wv3)
                for rtg in range(NB):
                    psums = [
                        pp.tile([128, 512], f32, tag=f"pp{j}", name=f"pp{j}")
                        for j in range(4)
                    ]
                    for ktg in range(4):
                        ht = hpool.tile([128, 8, 512], fp16, tag="ht", name="ht")
                        nc.sync.dma_start(
                            out=ht,
                            in_=hkv3[
                                :, ktg * 8 : (ktg + 1) * 8, rtg * 512 : (rtg + 1) * 512
                            ],
                        )
                        for kl in range(8):
                            kt = ktg * 8 + kl
                            for j in range(4):
                                nc.tensor.matmul(
                                    psums[j],
                                    ht[:, kl, j * 128 : (j + 1) * 128],
                                    whs[kt // 4][:, kt % 4, :],
                                    start=(kt == 0),
                                    stop=(kt == KT - 1),
                                )
                    if rtg == 2:
                        eb_dma(0)
                    for j in range(4):
                        o = v_s[:, rtg * 4 + j, :]
                        if j % 2 == 0:
                            nc.scalar.copy(out=o, in_=psums[j])
                        else:
                            nc.vector.tensor_copy(out=o, in_=psums[j])
                    if rtg == 3:
                        eb_dma(1)

            # ---------------- Phase B: attention + output projection ----------
            with (
                tc.tile_pool(name="wop", bufs=1) as wop,
                tc.tile_pool(name="sps", bufs=2, space="PSUM") as sps,
                tc.tile_pool(name="cps", bufs=1, space="PSUM") as cps,
                tc.tile_pool(name="ops", bufs=2, space="PSUM") as ops,
                tc.tile_pool(name="rps", bufs=1, space="PSUM") as rps,
            ):
                wo_s = wop.tile([128, HPC, D], fp16)  # 32KB/part
                nc.gpsimd.dma_start(out=wo_s, in_=wo3)
                ctxT_s = wop.tile([128, HPC, R], fp16)  # 16KB/part

                def scores_part(n, h, pT, trange):
                    b = n // 2
                    ebt = eb_tiles[n * 2 + h // 2]
                    for t in trange:
                        s = sps.tile([128, 1024], f32, tag="s", name="s")
                        for u in range(2):
                            kt = 2 * t + u
                            nc.tensor.matmul(
                                s[:, u * 512 : (u + 1) * 512],
                                kT_s[
                                    :,
                                    h,
                                    b * 1024 + kt * 128 : b * 1024 + (kt + 1) * 128,
                                ],
                                qT_s[:, h, n * 512 : (n + 1) * 512],
                                start=True,
                                stop=True,
                            )
                        es = epool.tile([128, 2, 512], fp16, tag="es", name="es")
                        nc.scalar.activation(es, s, Exp, bias=shift_b)
                        nc.vector.tensor_mul(
                            out=pT[:, 2 * t : 2 * t + 2, :],
                            in0=es,
                            in1=ebt[:, h % 2, 2 * t : 2 * t + 2, :],
                        )

                def tree(pT):
                    t4 = tpool.tile([128, 4, 512], fp16, tag="t4", name="t4")
                    nc.vector.tensor_add(t4, pT[:, 0:4, :], pT[:, 4:8, :])
                    t2 = tpool.tile([128, 2, 512], fp16, tag="t2", name="t2")
                    nc.vector.tensor_add(t2, t4[:, 0:2, :], t4[:, 2:4, :])
                    acc = apool.tile([128, 512], fp16, tag="acc", name="acc")
                    nc.vector.tensor_add(acc, t2[:, 0, :], t2[:, 1, :])
                    return acc

                def tail(n, h, pT, acc):
                    b = n // 2
                    rs = rps.tile([128, 512], f32, tag="r", name="rs")
                    nc.tensor.matmul(rs[0:1, :], ones_col, acc, start=True, stop=True)
                    rec = rpool.tile([1, 512], fp16, tag="rec", name="rec")
                    with nc.allow_low_precision(
                        reason="fp16 softmax denominators; validated vs reference"
                    ):
                        nc.vector.reciprocal(out=rec, in_=rs[0:1, :])
                    c = cps.tile([128, 512], f32, tag="c", name="c")
                    for kt in range(8):
                        nc.tensor.matmul(
                            c,
                            v_s[:, b * 8 + kt, h * 128 : (h + 1) * 128],
                            pT[:, kt, :],
                            start=(kt == 0),
                            stop=(kt == 7),
                        )
                    rb = rps.tile([128, 512], f32, tag="r", name="rb")
                    nc.tensor.matmul(rb, ones_row, rec, start=True, stop=True)
                    rb_sb = rpool.tile([128, 512], fp16, tag="rbs", name="rb_sb")
                    nc.vector.tensor_copy(out=rb_sb, in_=rb)
                    nc.vector.tensor_mul(
                        out=ctxT_s[:, h, n * 512 : (n + 1) * 512], in0=c, in1=rb_sb
                    )

                def outproj(n, mgs):
                    for mg in mgs:
                        osb = opool.tile([128, 8, 512], fp16, tag="osb", name="osb")
                        for j in range(8):
                            m = mg * 8 + j
                            o = ops.tile([128, 512], f32, tag="o", name="o")
                            for t in range(HPC):
                                nc.tensor.matmul(
                                    o,
                                    wo_s[:, t, m * 128 : (m + 1) * 128],
                                    ctxT_s[:, t, n * 512 : (n + 1) * 512],
                                    start=(t == 0),
                                    stop=(t == HPC - 1),
                                )
                            nc.scalar.copy(out=osb[:, j, :], in_=o)
                        dmae = nc.gpsimd if mg % 2 == 0 else nc.sync
                        dmae.dma_start(
                            out=outp[n, :, mg * 8 : (mg + 1) * 8, :], in_=osb
                        )

                seq = [(n, h) for n in range(NB) for h in range(HPC)]
                pend = {}
                for idx, (n, h) in enumerate(seq):
                    half = idx // 2
                    if h % 2 == 0 and half + 2 < 2 * NB:
                        eb_dma(half + 2)
                    pT = ppool.tile([128, 8, 512], fp16, tag="pT", name="pT")
                    scores_part(n, h, pT, (0, 1))
                    if idx >= 2:
                        tn, th = seq[idx - 2]
                        tail(tn, th, *pend.pop(idx - 2))
                    scores_part(n, h, pT, (2, 3))
                    pend[idx] = (pT, tree(pT))
                    if n > 0:
                        if h == 1:
                            outproj(n - 1, (0, 1))
                        elif h == 2:
                            outproj(n - 1, (2,))
                        elif h == 3:
                            outproj(n - 1, (3,))
                for idx in (len(seq) - 2, len(seq) - 1):
                    tn, th = seq[idx]
                    tail(tn, th, *pend.pop(idx))
                outproj(NB - 1, (0, 1, 2, 3))

    nc.compile()
    return nc


_NC_CACHE = None


def _prep_in_maps(hidden_q, hidden_kv, attention_mask, position_bias, Wq, Wk, Wv, Wo):
    hqT = np.ascontiguousarray(
        np.asarray(hidden_q, dtype=np.float32).reshape(R, D).T
    ).astype(np.float16)
    hkvT = np.ascontiguousarray(
        np.asarray(hidden_kv, dtype=np.float32).reshape(R, D).T
    ).astype(np.float16)
    mask = np.asarray(attention_mask)
    pb = np.asarray(position_bias, dtype=np.float32)

    # exp(position_bias) with the mask folded in as exact zeros
    ebf = np.exp(np.minimum(pb, 10.0)) * mask[:, None, :, :]  # [B, H, S, S]

    in_maps = []
    for c in range(NCORES):
        h0 = c * HPC
        e = ebf[:, h0 : h0 + HPC]  # [2, 4, 1024, 1024] (b, h, q, k)
        e = e.reshape(2, HPC, 2, 512, 8, 128)  # b, h, qb, qq, kt, p
        e = np.ascontiguousarray(e.transpose(0, 2, 1, 5, 4, 3))  # b, qb, h, p, kt, qq
        eb_host = e.reshape(NB, HPC, 128, KT * 128).astype(np.float16)
        in_maps.append(
            {
                "hqT": hqT,
                "hkvT": hkvT,
                "wq": np.ascontiguousarray(Wq[:, h0 * DH : (h0 + HPC) * DH]).astype(
                    np.float16
                ),
                "wk": np.ascontiguousarray(Wk[:, h0 * DH : (h0 + HPC) * DH]).astype(
                    np.float16
                ),
                "wv": np.ascontiguousarray(Wv[:, h0 * DH : (h0 + HPC) * DH]).astype(
                    np.float16
                ),
                "wo": np.ascontiguousarray(Wo[h0 * DH : (h0 + HPC) * DH, :]).astype(
                    np.float16
                ),
                "eb": eb_host,
            }
        )
    return in_maps


def kernel(
    hidden_q: np.ndarray,
    hidden_kv: np.ndarray,
    attention_mask: np.ndarray,
    position_bias: np.ndarray,
    Wq: np.ndarray,
    Wk: np.ndarray,
    Wv: np.ndarray,
    Wo: np.ndarray,
) -> np.ndarray:
    from concourse.bass_utils import run_bass_kernel_spmd

    global _NC_CACHE
    if _NC_CACHE is None:
        _NC_CACHE = _build_core_kernel()
    nc = _NC_CACHE

    in_maps = _prep_in_maps(
        hidden_q, hidden_kv, attention_mask, position_bias, Wq, Wk, Wv, Wo
    )
    res = run_bass_kernel_spmd(nc, in_maps, list(range(NCORES)))
    acc = res.results[0]["outp"].astype(np.float32)
    for c in range(1, NCORES):
        acc += res.results[c]["outp"]
    # acc [NB, 128, 32, 512]: value[n, p, m, c] = outT[m*128+p, n*512+c]
    out = np.ascontiguousarray(acc.transpose(0, 3, 2, 1)).reshape(R, D)
    return out.reshape(B, S, D)
